# revision 1
# baseline (speedup 1.0000x reference)
"""Multi-head attention (B=4, S=2048, D=1024, H=16, causal) on 8 trn2 cores.

Sharding: core c -> (batch b = c//2, head-group hg = c%2 of 8 heads).
Host pre-transposes/casts activations to bf16 [D, S] and slices weights;
device computes a partial [S, D] output (its head-group's contribution
through the output projection); host sums the pair per batch and adds bo.
"""

import numpy as np
import ml_dtypes

import concourse.bacc as bacc
import concourse.bass as bass
import concourse.mybir as mybir
import concourse.tile as tile
from concourse.bass_utils import run_bass_kernel_spmd

B, S, D, H = 4, 2048, 1024, 16
DH = D // H          # 64
HG = H // 2          # 8 heads per core
DG = HG * DH         # 512 dims per core
N_CORES = 8

BF16 = mybir.dt.bfloat16
F32 = mybir.dt.float32

ST = S // 128        # 16 seq tiles of 128
QB = S // 512        # 4 query blocks of 512
KT = D // 128        # 8 contraction tiles for the input projections
VBLK = DH + 1        # 65: per-head v columns + ones column
AF = mybir.ActivationFunctionType
ALU = mybir.AluOpType


def build_program(loop_r=0):
    """loop_r > 0 builds a measurement variant that repeats the whole body
    loop_r times inside an on-device loop (for timing via slope)."""
    nc = bacc.Bacc("TRN2", target_bir_lowering=False, debug=False,
                   num_devices=N_CORES)

    xq = nc.declare_dram_parameter("xq", [D, S], BF16, isOutput=False)
    xk = nc.declare_dram_parameter("xk", [D, S], BF16, isOutput=False)
    xv = nc.declare_dram_parameter("xv", [D, S], BF16, isOutput=False)
    wq = nc.declare_dram_parameter("wq", [D, DG], BF16, isOutput=False)
    wk = nc.declare_dram_parameter("wk", [D, DG], BF16, isOutput=False)
    wv = nc.declare_dram_parameter("wv", [D, DG], BF16, isOutput=False)
    wo = nc.declare_dram_parameter("wo", [DG, D], BF16, isOutput=False)
    bq = nc.declare_dram_parameter("bq", [DG, 1], F32, isOutput=False)
    bk = nc.declare_dram_parameter("bk", [DG, 1], F32, isOutput=False)
    bv = nc.declare_dram_parameter("bv", [DG, 1], F32, isOutput=False)
    out = nc.declare_dram_parameter("out", [S, D], F32, isOutput=True)

    with tile.TileContext(nc) as tc:
        with (
            tc.tile_pool(name="persist", bufs=1) as persist,
            tc.tile_pool(name="xin", bufs=4) as xin,
            tc.tile_pool(name="xvin", bufs=1) as xvin,
            tc.tile_pool(name="exp", bufs=8) as expp,
            tc.tile_pool(name="small", bufs=3) as small,
            tc.tile_pool(name="outp", bufs=2) as outp,
            tc.tile_pool(name="ps512", bufs=2, space="PSUM") as ps512,
            tc.tile_pool(name="pssc", bufs=2, space="PSUM") as pssc,
            tc.tile_pool(name="psav", bufs=2, space="PSUM") as psav,
        ):
            import contextlib
            loop_cm = tc.For_i(0, loop_r, 1) if loop_r else contextlib.nullcontext()
            with loop_cm:
                emit_body(nc, tc, locals())
    nc.compile()
    return nc


def emit_body(nc, tc, pools):
    persist = pools["persist"]; xin = pools["xin"]; xvin = pools["xvin"]
    expp = pools["expp"]; small = pools["small"]; outp = pools["outp"]
    ps512 = pools["ps512"]; pssc = pools["pssc"]; psav = pools["psav"]
    xq = pools["xq"]; xk = pools["xk"]; xv = pools["xv"]
    wq = pools["wq"]; wk = pools["wk"]; wv = pools["wv"]; wo = pools["wo"]
    bq = pools["bq"]; bk = pools["bk"]; bv = pools["bv"]; out = pools["out"]
    if True:
        if True:
            # ---- resident weights / constants (k-proj inputs first) ----
            wq_sb = persist.tile([128, KT * DG], BF16, tag="wq")
            wk_sb = persist.tile([128, KT * DG], BF16, tag="wk")
            wv_sb = persist.tile([128, KT * DG], BF16, tag="wv")
            wo_sb = persist.tile([128, 4 * D], BF16, tag="wo")
            bq_sb = persist.tile([128, 4], F32, tag="bq")
            bk_sb = persist.tile([128, 4], F32, tag="bk")
            bv_sb = persist.tile([128, 4], F32, tag="bv")
            nc.sync.dma_start(
                wk_sb[:].rearrange("p (j c) -> p j c", j=KT),
                wk[:].rearrange("(j p) c -> p j c", p=128),
            )
            nc.sync.dma_start(
                bk_sb[:].rearrange("p (t o) -> p t o", o=1),
                bk[:].rearrange("(t p) o -> p t o", p=128),
            )

            # two tril mask tiles side by side: mask[p, f] = 1.0 if p <= f%128
            masks = persist.tile([128, 256], BF16, tag="masks")
            nc.gpsimd.memset(masks[:], 1.0)
            for mi in range(2):
                nc.gpsimd.affine_select(
                    out=masks[:, bass.ts(mi, 128)],
                    in_=masks[:, bass.ts(mi, 128)],
                    compare_op=ALU.is_ge,
                    fill=0.0,
                    base=0,
                    pattern=[[1, 128]],
                    channel_multiplier=-1,
                )
            ones64 = persist.tile([1, DH], F32, tag="ones64")
            nc.gpsimd.memset(ones64[:], 1.0)

            # persistent activations
            qt = [persist.tile([128, S], BF16, tag=f"qt{t}", name=f"qt{t}") for t in range(4)]
            kt = [persist.tile([128, S], BF16, tag=f"kt{t}", name=f"kt{t}") for t in range(4)]
            v_sb = persist.tile([128, ST * HG * VBLK], BF16, tag="v_sb")
            ao = [persist.tile([128, S], BF16, tag=f"ao{t}", name=f"ao{t}") for t in range(4)]

            # ones columns of v blocks (written before the v copies below)
            v_view = v_sb[:].rearrange("p (s h c) -> p s h c", s=ST, h=HG, c=VBLK)
            nc.gpsimd.memset(v_view[:, :, :, DH : DH + 1], 1.0)

            # input chunk prefetch helper (one DMA per tensor per 512-block)
            chunk_tiles = {}

            def prefetch_chunk(n):
                xk_sb = xin.tile([128, KT * 512], BF16, tag="xkq", name=f"xk_sb{n}")
                nc.sync.dma_start(
                    xk_sb[:].rearrange("p (j c) -> p j c", j=KT),
                    xk[:, bass.ts(n, 512)].rearrange("(j p) c -> p j c", p=128),
                )
                xq_sb = xin.tile([128, KT * 512], BF16, tag="xkq", name=f"xq_sb{n}")
                nc.sync.dma_start(
                    xq_sb[:].rearrange("p (j c) -> p j c", j=KT),
                    xq[:, bass.ts(n, 512)].rearrange("(j p) c -> p j c", p=128),
                )
                chunk_tiles[n] = (xk_sb, xq_sb)

            prefetch_chunk(0)
            # remaining weights, then the full v input
            nc.sync.dma_start(
                wq_sb[:].rearrange("p (j c) -> p j c", j=KT),
                wq[:].rearrange("(j p) c -> p j c", p=128),
            )
            nc.sync.dma_start(
                bq_sb[:].rearrange("p (t o) -> p t o", o=1),
                bq[:].rearrange("(t p) o -> p t o", p=128),
            )
            nc.sync.dma_start(
                wv_sb[:].rearrange("p (j c) -> p j c", j=KT),
                wv[:].rearrange("(j p) c -> p j c", p=128),
            )
            nc.sync.dma_start(
                bv_sb[:].rearrange("p (t o) -> p t o", o=1),
                bv[:].rearrange("(t p) o -> p t o", p=128),
            )
            xv_sb = xvin.tile([128, KT * S], BF16, tag="xv")
            nc.sync.dma_start(
                xv_sb[:].rearrange("p (j c) -> p j c", j=KT),
                xv[:].rearrange("(j p) c -> p j c", p=128),
            )
            xv_t = [xv_sb[:, bass.ts(j, S)] for j in range(KT)]
            nc.sync.dma_start(
                wo_sb[:].rearrange("p (j c) -> p j c", j=4),
                wo[:].rearrange("(j p) c -> p j c", p=128),
            )

            def emit_kproj(n, t):
                xk_sb, _ = chunk_tiles[n]
                ps = ps512.tile([128, 512], F32, tag="mm512", name="psk")
                for j in range(KT):
                    nc.tensor.matmul(
                        ps[:],
                        wk_sb[:, j * DG + t * 128 : j * DG + (t + 1) * 128],
                        xk_sb[:, bass.ts(j, 512)],
                        start=(j == 0),
                        stop=(j == KT - 1),
                    )
                nc.vector.tensor_scalar_add(
                    kt[t][:, bass.ts(n, 512)], ps[:], bk_sb[:, t : t + 1]
                )

            def emit_qproj(n, t):
                _, xq_sb = chunk_tiles[n]
                ps = ps512.tile([128, 512], F32, tag="mm512", name="psq")
                for j in range(KT):
                    nc.tensor.matmul(
                        ps[:],
                        wq_sb[:, j * DG + t * 128 : j * DG + (t + 1) * 128],
                        xq_sb[:, bass.ts(j, 512)],
                        start=(j == 0),
                        stop=(j == KT - 1),
                    )
                nc.vector.tensor_scalar(
                    qt[t][:, bass.ts(n, 512)], ps[:],
                    bq_sb[:, t : t + 1], 0.125, ALU.add, ALU.mult,
                )

            def emit_vproj(s):
                ps = ps512.tile([128, 512], F32, tag="mm512", name="psv")
                for j in range(KT):
                    nc.tensor.matmul(
                        ps[:],
                        xv_t[j][:, bass.ts(s, 128)],
                        wv_sb[:, bass.ts(j, DG)],
                        start=(j == 0),
                        stop=(j == KT - 1),
                    )
                nc.vector.tensor_copy(
                    v_view[:, s, :, 0:DH], ps[:].rearrange("p (h c) -> p h c", c=DH)
                )

            def emit_oproj(s):
                ob = outp.tile([128, 1024], F32, tag="ob", name="ob")
                for m in range(2):
                    po = ps512.tile([128, 512], F32, tag="mm512", name="po")
                    for kk in range(4):
                        nc.tensor.matmul(
                            po[:],
                            ao[kk][:, bass.ts(s, 128)],
                            wo_sb[:, kk * D + m * 512 : kk * D + (m + 1) * 512],
                            start=(kk == 0),
                            stop=(kk == 3),
                        )
                    nc.vector.tensor_copy(ob[:, bass.ts(m, 512)], po[:])
                nc.sync.dma_start(out[bass.ts(s, 128), :], ob[:])

            def proj_block(n):
                for t in range(4):
                    emit_kproj(n, t)
                for t in range(4):
                    emit_qproj(n, t)
                for s in range(4 * n, 4 * n + 4):
                    emit_vproj(s)

            proj_block(0)
            for n in range(QB):
                if n + 1 < QB:
                    prefetch_chunk(n + 1)
                    bg = (
                        [lambda t=t: emit_kproj(n + 1, t) for t in range(4)]
                        + [lambda t=t: emit_qproj(n + 1, t) for t in range(4)]
                        + [lambda s=s: emit_vproj(s) for s in range(4 * n + 4, 4 * n + 8)]
                    )
                else:
                    bg = []
                if n == 1:
                    bg += [lambda s=s: emit_oproj(s) for s in range(0, 4)]
                elif n == 3:
                    bg += [lambda s=s: emit_oproj(s) for s in range(4, 12)]
                # attention for q block n
                def finalize(entry):
                    av_, recip_, t_, r_, n_ = entry
                    bc = ps512.tile([128, 512], F32, tag="mm512", name="bc")
                    nc.tensor.matmul(
                        bc[0:DH, :], ones64[:], recip_[:], start=True, stop=True
                    )
                    rb = small.tile([DH, 512], F32, tag="rb", name="rb")
                    nc.vector.tensor_copy(rb[:], bc[0:DH, :])
                    dst = ao[t_][r_ * DH : (r_ + 1) * DH, bass.ts(n_, 512)]
                    nc.vector.tensor_mul(dst, av_[0:DH, :], rb[:])
                    nc.vector.tensor_scalar_add(
                        dst, dst, bv_sb[r_ * DH : (r_ + 1) * DH, t_ : t_ + 1]
                    )

                nk = 4 * (n + 1)

                def make_full_unit(h, av, j0):
                    t, r = h // 2, h % 2
                    q_ap = qt[t][r * DH : (r + 1) * DH, bass.ts(n, 512)]
                    ex_box = []

                    def stage1():
                        sc = pssc.tile([128, 1024], F32, tag="sc", name="sc")
                        for d in range(2):
                            nc.tensor.matmul(
                                sc[:, bass.ts(d, 512)],
                                kt[t][r * DH : (r + 1) * DH, bass.ts(j0 + d, 128)],
                                q_ap,
                                start=True,
                                stop=True,
                            )
                        ex = expp.tile([128, 1024], BF16, tag="ex", name="ex")
                        nc.scalar.activation(ex[:], sc[:], AF.Exp)
                        ex_box.append(ex)

                    def stage2():
                        ex = ex_box[0]
                        for d in range(2):
                            j = j0 + d
                            nc.tensor.matmul(
                                av[:],
                                v_sb[:, (j * HG * VBLK + h * VBLK) : (j * HG * VBLK + h * VBLK) + VBLK],
                                ex[:, bass.ts(d, 512)],
                                start=(j == 0),
                                stop=False,
                            )

                    return stage1, stage2

                def make_band_unit(h, av, rp):
                    t, r = h // 2, h % 2
                    q_ap = qt[t][r * DH : (r + 1) * DH, bass.ts(n, 512)]
                    r0, r1 = 2 * rp, 2 * rp + 1
                    nw0, nw1 = 512 - 128 * r0, 512 - 128 * r1
                    ex_box = []

                    def stage1():
                        sc = pssc.tile([128, 1024], F32, tag="sc", name="scb")
                        for ri, off, nw in ((r0, 0, nw0), (r1, nw0, nw1)):
                            nc.tensor.matmul(
                                sc[:, off : off + nw],
                                kt[t][r * DH : (r + 1) * DH, bass.ts(4 * n + ri, 128)],
                                q_ap[:, 128 * ri : 512],
                                start=True,
                                stop=True,
                            )
                        ex = expp.tile([128, 1024], BF16, tag="ex", name="exb")
                        nc.scalar.activation(
                            ex[:, 0 : nw0 + nw1], sc[:, 0 : nw0 + nw1], AF.Exp
                        )
                        nc.vector.tensor_mul(
                            ex[:, 0:128], ex[:, 0:128], masks[:, 0:128]
                        )
                        nc.vector.tensor_mul(
                            ex[:, nw0 : nw0 + 128], ex[:, nw0 : nw0 + 128],
                            masks[:, 128:256],
                        )
                        ex_box.append(ex)

                    def stage2():
                        ex = ex_box[0]
                        for ri, off, nw in ((r0, 0, nw0), (r1, nw0, nw1)):
                            j = 4 * n + ri
                            nc.tensor.matmul(
                                av[:, 128 * ri : 512],
                                v_sb[:, (j * HG * VBLK + h * VBLK) : (j * HG * VBLK + h * VBLK) + VBLK],
                                ex[:, off : off + nw],
                                start=(j == 0),
                                stop=(j == nk - 1),
                            )

                    return stage1, stage2

                for hp in range(0, HG, 2):
                    hA, hB = hp, hp + 1
                    avA = psav.tile([VBLK, 512], F32, tag="av", name="avA")
                    avB = psav.tile([VBLK, 512], F32, tag="av", name="avB")
                    units = []
                    for j0 in range(0, 4 * n, 2):
                        units.append(make_full_unit(hA, avA, j0))
                        units.append(make_full_unit(hB, avB, j0))
                    for rp in range(2):
                        units.append(make_band_unit(hA, avA, rp))
                        units.append(make_band_unit(hB, avB, rp))
                    # skew-1 software pipeline: sc/exp of unit u+1 before av of u
                    prev = None
                    for ui, (s1, s2) in enumerate(units):
                        s1()
                        if prev is not None:
                            prev()
                        elif bg:
                            bg.pop(0)()
                        prev = units[ui][1]
                    prev()
                    recipA = small.tile([1, 512], F32, tag="recip", name="recipA")
                    nc.vector.reciprocal(recipA[:], avA[DH : DH + 1, :])
                    recipB = small.tile([1, 512], F32, tag="recip", name="recipB")
                    nc.vector.reciprocal(recipB[:], avB[DH : DH + 1, :])
                    if bg:
                        bg.pop(0)()
                    finalize((avA, recipA, hp // 2, 0, n))
                    if bg:
                        bg.pop(0)()
                    finalize((avB, recipB, hp // 2, 1, n))
                    if bg:
                        bg.pop(0)()
                while bg:
                    bg.pop(0)()
                # output projection of this block runs inside the next
                # block's attention (bg queue); last block emits directly.
                if n == QB - 1:
                    for s in range(4 * n, 4 * n + 4):
                        emit_oproj(s)


_NC = None


def _get_program():
    global _NC
    if _NC is None:
        _NC = build_program()
    return _NC


def make_in_maps(query, key, value, Wq, bq, Wk, bk, Wv, bv, Wo):
    bf = ml_dtypes.bfloat16
    in_maps = []
    xqs = [np.ascontiguousarray(query[b].T).astype(bf) for b in range(B)]
    xks = [np.ascontiguousarray(key[b].T).astype(bf) for b in range(B)]
    xvs = [np.ascontiguousarray(value[b].T).astype(bf) for b in range(B)]
    for c in range(N_CORES):
        b, hg = c // 2, c % 2
        sl = slice(hg * DG, (hg + 1) * DG)
        in_maps.append({
            "xq": xqs[b], "xk": xks[b], "xv": xvs[b],
            "wq": np.ascontiguousarray(Wq[sl, :].T).astype(bf),
            "wk": np.ascontiguousarray(Wk[sl, :].T).astype(bf),
            "wv": np.ascontiguousarray(Wv[sl, :].T).astype(bf),
            "wo": np.ascontiguousarray(Wo[:, sl].T).astype(bf),
            "bq": np.asarray(bq[sl], np.float32).reshape(DG, 1),
            "bk": np.asarray(bk[sl], np.float32).reshape(DG, 1),
            "bv": np.asarray(bv[sl], np.float32).reshape(DG, 1),
        })
    return in_maps


def combine_outputs(results, bo):
    out = np.empty((B, S, D), np.float32)
    for b in range(B):
        out[b] = results[2 * b]["out"] + results[2 * b + 1]["out"]
        out[b] += np.asarray(bo, np.float32)[None, :]
    return out


def kernel(query, key, value, mask, Wq, bq, Wk, bk, Wv, bv, Wo, bo):
    # mask is the causal tril mask from the reference problem; causality is
    # implemented directly in the device kernel.
    nc = _get_program()
    in_maps = make_in_maps(
        np.asarray(query, np.float32), np.asarray(key, np.float32),
        np.asarray(value, np.float32), np.asarray(Wq, np.float32),
        np.asarray(bq, np.float32), np.asarray(Wk, np.float32),
        np.asarray(bk, np.float32), np.asarray(Wv, np.float32),
        np.asarray(bv, np.float32), np.asarray(Wo, np.float32),
    )
    res = run_bass_kernel_spmd(nc, in_maps, list(range(N_CORES)))
    return combine_outputs(res.results, np.asarray(bo, np.float32))



# revision 13
# speedup vs baseline: 2.0230x; 2.0230x over previous
"""Multi-head attention (B=4, S=2048, D=1024, H=16, causal) on 8 trn2 cores.

Sharding: core c -> (batch b = c//2, head-group hg = c%2 of 8 heads).
Host pre-transposes/casts activations to bf16 [D, S] and slices weights;
device computes a partial [S, D] output (its head-group's contribution
through the output projection); host sums the pair per batch and adds bo.
"""

import numpy as np
import ml_dtypes

import concourse.bacc as bacc
import concourse.bass as bass
import concourse.mybir as mybir
import concourse.tile as tile
from concourse.bass_utils import run_bass_kernel_spmd

B, S, D, H = 4, 2048, 1024, 16
DH = D // H          # 64
HG = H // 2          # 8 heads per core
DG = HG * DH         # 512 dims per core
N_CORES = 8

BF16 = mybir.dt.bfloat16
F32 = mybir.dt.float32

ST = S // 128        # 16 seq tiles of 128
QB = S // 512        # 4 query blocks of 512
KT = D // 128        # 8 contraction tiles for the input projections
VBLK = DH + 1        # 65: per-head v columns + ones column
AF = mybir.ActivationFunctionType
ALU = mybir.AluOpType


def build_program(loop_r=0):
    """loop_r > 0 builds a measurement variant that repeats the whole body
    loop_r times inside an on-device loop (for timing via slope)."""
    nc = bacc.Bacc("TRN2", target_bir_lowering=False, debug=False,
                   num_devices=N_CORES)

    xq = nc.declare_dram_parameter("xq", [D, S], BF16, isOutput=False)
    xk = nc.declare_dram_parameter("xk", [D, S], BF16, isOutput=False)
    xv = nc.declare_dram_parameter("xv", [D, S], BF16, isOutput=False)
    wq = nc.declare_dram_parameter("wq", [D, DG], BF16, isOutput=False)
    wk = nc.declare_dram_parameter("wk", [D, DG], BF16, isOutput=False)
    wv = nc.declare_dram_parameter("wv", [D, DG], BF16, isOutput=False)
    wo = nc.declare_dram_parameter("wo", [DG, D], BF16, isOutput=False)
    bq = nc.declare_dram_parameter("bq", [DG, 1], F32, isOutput=False)
    bk = nc.declare_dram_parameter("bk", [DG, 1], F32, isOutput=False)
    bv = nc.declare_dram_parameter("bv", [DG, 1], F32, isOutput=False)
    out = nc.declare_dram_parameter("out", [S, D], F32, isOutput=True)

    with tile.TileContext(nc) as tc:
        with (
            tc.tile_pool(name="persist", bufs=1) as persist,
            tc.tile_pool(name="xin", bufs=4) as xin,
            tc.tile_pool(name="xvin", bufs=1) as xvin,
            tc.tile_pool(name="exp", bufs=8) as expp,
            tc.tile_pool(name="small", bufs=2) as small,
            tc.tile_pool(name="outp", bufs=2) as outp,
            tc.tile_pool(name="ps512", bufs=2, space="PSUM") as ps512,
            tc.tile_pool(name="pssc", bufs=2, space="PSUM") as pssc,
            tc.tile_pool(name="psav", bufs=2, space="PSUM") as psav,
        ):
            import contextlib
            loop_cm = tc.For_i(0, loop_r, 1) if loop_r else contextlib.nullcontext()
            with loop_cm:
                emit_body(nc, tc, locals())
    nc.compile()
    return nc


def emit_body(nc, tc, pools):
    persist = pools["persist"]; xin = pools["xin"]; xvin = pools["xvin"]
    expp = pools["expp"]; small = pools["small"]; outp = pools["outp"]
    ps512 = pools["ps512"]; pssc = pools["pssc"]; psav = pools["psav"]
    xq = pools["xq"]; xk = pools["xk"]; xv = pools["xv"]
    wq = pools["wq"]; wk = pools["wk"]; wv = pools["wv"]; wo = pools["wo"]
    bq = pools["bq"]; bk = pools["bk"]; bv = pools["bv"]; out = pools["out"]
    if True:
        if True:
            # ---- resident weights / constants (k-proj inputs first) ----
            wq_sb = persist.tile([128, KT * DG], BF16, tag="wq")
            wk_sb = persist.tile([128, KT * DG], BF16, tag="wk")
            wv_sb = persist.tile([128, KT * DG], BF16, tag="wv")
            wo_sb = persist.tile([128, 4 * D], BF16, tag="wo")
            bq_sb = persist.tile([128, 4], F32, tag="bq")
            bk_sb = persist.tile([128, 4], F32, tag="bk")
            bv_sb = persist.tile([128, 4], F32, tag="bv")
            nc.scalar.dma_start(
                wk_sb[:].rearrange("p (j c) -> p j c", j=KT),
                wk[:].rearrange("(j p) c -> p j c", p=128),
            )
            nc.scalar.dma_start(
                bk_sb[:].rearrange("p (t o) -> p t o", o=1),
                bk[:].rearrange("(t p) o -> p t o", p=128),
            )

            # two tril mask tiles side by side: mask[p, f] = 1.0 if p <= f%128
            masks = persist.tile([128, 256], BF16, tag="masks")
            nc.gpsimd.memset(masks[:], 1.0)
            for mi in range(2):
                nc.gpsimd.affine_select(
                    out=masks[:, bass.ts(mi, 128)],
                    in_=masks[:, bass.ts(mi, 128)],
                    compare_op=ALU.is_ge,
                    fill=0.0,
                    base=0,
                    pattern=[[1, 128]],
                    channel_multiplier=-1,
                )
            ones64 = persist.tile([1, DH], F32, tag="ones64")
            nc.gpsimd.memset(ones64[:], 1.0)

            # persistent activations
            qt = [persist.tile([128, S], BF16, tag=f"qt{t}", name=f"qt{t}") for t in range(4)]
            kt = [persist.tile([128, S], BF16, tag=f"kt{t}", name=f"kt{t}") for t in range(4)]
            v_sb = persist.tile([128, ST * HG * VBLK], BF16, tag="v_sb")
            ao = [persist.tile([128, S], BF16, tag=f"ao{t}", name=f"ao{t}") for t in range(4)]

            # ones columns of v blocks (written before the v copies below)
            v_view = v_sb[:].rearrange("p (s h c) -> p s h c", s=ST, h=HG, c=VBLK)
            nc.gpsimd.memset(v_view[:, :, :, DH : DH + 1], 1.0)

            # input chunk prefetch helper (one DMA per tensor per 512-block)
            chunk_tiles = {}

            def prefetch_chunk(n):
                xk_sb = xin.tile([128, KT * 512], BF16, tag="xkq", name=f"xk_sb{n}")
                nc.sync.dma_start(
                    xk_sb[:].rearrange("p (j c) -> p j c", j=KT),
                    xk[:, bass.ts(n, 512)].rearrange("(j p) c -> p j c", p=128),
                )
                xq_sb = xin.tile([128, KT * 512], BF16, tag="xkq", name=f"xq_sb{n}")
                nc.sync.dma_start(
                    xq_sb[:].rearrange("p (j c) -> p j c", j=KT),
                    xq[:, bass.ts(n, 512)].rearrange("(j p) c -> p j c", p=128),
                )
                chunk_tiles[n] = (xk_sb, xq_sb)

            prefetch_chunk(0)
            # remaining weights, then the full v input
            nc.scalar.dma_start(
                wq_sb[:].rearrange("p (j c) -> p j c", j=KT),
                wq[:].rearrange("(j p) c -> p j c", p=128),
            )
            nc.scalar.dma_start(
                bq_sb[:].rearrange("p (t o) -> p t o", o=1),
                bq[:].rearrange("(t p) o -> p t o", p=128),
            )
            nc.scalar.dma_start(
                wv_sb[:].rearrange("p (j c) -> p j c", j=KT),
                wv[:].rearrange("(j p) c -> p j c", p=128),
            )
            nc.scalar.dma_start(
                bv_sb[:].rearrange("p (t o) -> p t o", o=1),
                bv[:].rearrange("(t p) o -> p t o", p=128),
            )
            # xv arrives in four 512-column chunks so vproj of block 0 can
            # start early and the Sync queue isn't blocked by one huge
            # descriptor-generation pass.
            xv_sb = xvin.tile([128, KT * S], BF16, tag="xv")
            xv_view = xv_sb[:].rearrange("p (j nn c) -> p j nn c", j=KT, nn=QB)
            for nn in range(QB):
                nc.sync.dma_start(
                    xv_view[:, :, nn, :],
                    xv[:, bass.ts(nn, 512)].rearrange("(j p) c -> p j c", p=128),
                )
            xv_t = [xv_sb[:, bass.ts(j, S)] for j in range(KT)]
            nc.scalar.dma_start(
                wo_sb[:].rearrange("p (j c) -> p j c", j=4),
                wo[:].rearrange("(j p) c -> p j c", p=128),
            )
            # warm up the PE clock (HAM) with dummy matmuls on the masks
            # tile while the first input DMAs land.
            for _ in range(30):
                wps = ps512.tile([128, 512], F32, tag="mm512", name="warm")
                nc.tensor.matmul(
                    wps[:, 0:256], masks[:, 0:128], masks[:], start=True,
                    stop=True,
                )

            def emit_kproj(n, t):
                xk_sb, _ = chunk_tiles[n]
                ps = ps512.tile([128, 512], F32, tag="mm512", name="psk")
                for j in range(KT):
                    nc.tensor.matmul(
                        ps[:],
                        wk_sb[:, j * DG + t * 128 : j * DG + (t + 1) * 128],
                        xk_sb[:, bass.ts(j, 512)],
                        start=(j == 0),
                        stop=(j == KT - 1),
                    )
                nc.vector.tensor_scalar_add(
                    kt[t][:, bass.ts(n, 512)], ps[:], bk_sb[:, t : t + 1]
                )

            def emit_qproj(n, t):
                _, xq_sb = chunk_tiles[n]
                ps = ps512.tile([128, 512], F32, tag="mm512", name="psq")
                for j in range(KT):
                    nc.tensor.matmul(
                        ps[:],
                        wq_sb[:, j * DG + t * 128 : j * DG + (t + 1) * 128],
                        xq_sb[:, bass.ts(j, 512)],
                        start=(j == 0),
                        stop=(j == KT - 1),
                    )
                nc.vector.tensor_scalar(
                    qt[t][:, bass.ts(n, 512)], ps[:],
                    bq_sb[:, t : t + 1], 0.125, ALU.add, ALU.mult,
                )

            def emit_vproj(s):
                ps = ps512.tile([128, 512], F32, tag="mm512", name="psv")
                for j in range(KT):
                    nc.tensor.matmul(
                        ps[:],
                        xv_t[j][:, bass.ts(s, 128)],
                        wv_sb[:, bass.ts(j, DG)],
                        start=(j == 0),
                        stop=(j == KT - 1),
                    )
                nc.vector.tensor_copy(
                    v_view[:, s, :, 0:DH], ps[:].rearrange("p (h c) -> p h c", c=DH)
                )

            def emit_oproj(s):
                ob = outp.tile([128, 1024], F32, tag="ob", name="ob")
                for m in range(2):
                    po = ps512.tile([128, 512], F32, tag="mm512", name="po")
                    for kk in range(4):
                        nc.tensor.matmul(
                            po[:],
                            ao[kk][:, bass.ts(s, 128)],
                            wo_sb[:, kk * D + m * 512 : kk * D + (m + 1) * 512],
                            start=(kk == 0),
                            stop=(kk == 3),
                        )
                    nc.vector.tensor_copy(ob[:, bass.ts(m, 512)], po[:])
                nc.sync.dma_start(out[bass.ts(s, 128), :], ob[:])

            def proj_block(n):
                for t in range(4):
                    emit_kproj(n, t)
                for t in range(4):
                    emit_qproj(n, t)
                for s in range(4 * n, 4 * n + 4):
                    emit_vproj(s)

            proj_block(0)
            for n in range(QB):
                if n + 1 < QB:
                    prefetch_chunk(n + 1)
                    bg = (
                        [lambda t=t: emit_kproj(n + 1, t) for t in range(4)]
                        + [lambda t=t: emit_qproj(n + 1, t) for t in range(4)]
                        + [lambda s=s: emit_vproj(s) for s in range(4 * n + 4, 4 * n + 8)]
                    )
                else:
                    bg = []
                if n == 1:
                    bg += [lambda s=s: emit_oproj(s) for s in range(0, 4)]
                elif n == 3:
                    bg += [lambda s=s: emit_oproj(s) for s in range(4, 12)]
                # pace bg pops evenly across the block's unit slots so the
                # next block's projections neither race their input DMA
                # (stalling the in-order PE queue) nor pile up at block end
                slots_total = max(1, 4 * 4 * (n + 1))
                stride = max(1, slots_total // max(1, len(bg)))
                slot_box = [0]

                def bg_tick():
                    s_ = slot_box[0]
                    slot_box[0] += 1
                    if bg and s_ % stride == stride - 1:
                        bg.pop(0)()

                # attention for q block n
                def finalize(entry):
                    # normalize both heads of the pair: the reciprocal rows
                    # are replicated across 64 partitions on the (idle)
                    # GPSIMD engine, then two DVE muls apply them.
                    avA_, avB_, recipA_, recipB_, t_, n_ = entry
                    rbA = small.tile([DH, 512], F32, tag="rbA", name="rbA")
                    nc.gpsimd.partition_broadcast(rbA[:], recipA_[:])
                    rbB = small.tile([DH, 512], F32, tag="rbB", name="rbB")
                    nc.gpsimd.partition_broadcast(rbB[:], recipB_[:])
                    for r_, av_, rb_ in ((0, avA_, rbA), (1, avB_, rbB)):
                        dst = ao[t_][r_ * DH : (r_ + 1) * DH, bass.ts(n_, 512)]
                        nc.vector.tensor_mul(dst, av_[0:DH, :], rb_[:])
                        nc.vector.tensor_scalar_add(
                            dst, dst, bv_sb[r_ * DH : (r_ + 1) * DH, t_ : t_ + 1]
                        )

                nk = 4 * (n + 1)

                def make_pair_unit(hA, hB, avA, avB, j):
                    # one k-tile, BOTH heads: the two sc matmuls sit at PE
                    # array rows 0-63 / 64-127 (row-tiled, issued
                    # back-to-back -> concurrent), write the two halves of
                    # one [128,1024] PSUM tile, and share one exp ACTIVATE.
                    t = hA // 2
                    is_band = j >= 4 * n
                    r = j - 4 * n if is_band else 0
                    nw = 512 - 128 * r
                    qoff = 512 * n + 128 * r
                    ex_box = []

                    def stage1():
                        sc = pssc.tile([128, 1024], F32, tag="sc", name="sc")
                        for half in range(2):
                            nc.tensor.matmul(
                                sc[:, 512 * half : 512 * half + nw],
                                kt[t][half * DH : (half + 1) * DH, bass.ts(j, 128)],
                                qt[t][half * DH : (half + 1) * DH,
                                      qoff : 512 * (n + 1)],
                                start=True,
                                stop=True,
                            )
                        ex = expp.tile([128, 1024], BF16, tag="ex", name="ex")
                        if nw == 512:
                            nc.scalar.activation(ex[:], sc[:], AF.Exp)
                        else:
                            nc.scalar.activation(
                                ex[:].rearrange("p (g c) -> p g c", g=2)[:, :, 0:nw],
                                sc[:].rearrange("p (g c) -> p g c", g=2)[:, :, 0:nw],
                                AF.Exp,
                            )
                        if is_band:
                            nc.vector.tensor_mul(
                                ex[:, 0:128], ex[:, 0:128], masks[:, 0:128]
                            )
                            nc.vector.tensor_mul(
                                ex[:, 512:640], ex[:, 512:640], masks[:, 128:256]
                            )
                        ex_box.append(ex)

                    def stage2():
                        ex = ex_box[0]
                        for half, (h, av) in enumerate(((hA, avA), (hB, avB))):
                            nc.tensor.matmul(
                                av[:, 128 * r : 512] if is_band else av[:],
                                v_sb[:, (j * HG + h) * VBLK : (j * HG + h + 1) * VBLK],
                                ex[:, 512 * half : 512 * half + nw],
                                start=(j == 0),
                                stop=(j == nk - 1),
                            )

                    return stage1, stage2

                for hp in range(0, HG, 2):
                    hA, hB = hp, hp + 1
                    avA = psav.tile([VBLK, 512], F32, tag="av", name="avA")
                    avB = psav.tile([VBLK, 512], F32, tag="av", name="avB")
                    units = [
                        make_pair_unit(hA, hB, avA, avB, j) for j in range(nk)
                    ]
                    # skew-1 software pipeline: sc/exp of unit u+1 before av
                    # of u; one bg (projection) item per unit keeps the PE
                    # duty cycle high so HAM stays at full clock.
                    prev = None
                    for ui, (s1, s2) in enumerate(units):
                        s1()
                        if prev is not None:
                            prev()
                        bg_tick()
                        prev = units[ui][1]
                    prev()
                    # den rows live at PSUM partition 64; the custom-DVE
                    # reciprocal only works at base partition 0, so copy out
                    # first (standard DVE copy handles the partition shift).
                    cpA = small.tile([1, 512], F32, tag="cp", name="cpA")
                    nc.vector.tensor_copy(cpA[:], avA[DH : DH + 1, :])
                    cpB = small.tile([1, 512], F32, tag="cp", name="cpB")
                    nc.vector.tensor_copy(cpB[:], avB[DH : DH + 1, :])
                    recipA = small.tile([1, 512], F32, tag="recip", name="recipA")
                    nc.vector.reciprocal_approx_fast(recipA[:], cpA[:])
                    recipB = small.tile([1, 512], F32, tag="recip", name="recipB")
                    nc.vector.reciprocal_approx_fast(recipB[:], cpB[:])
                    finalize((avA, avB, recipA, recipB, hp // 2, n))
                while bg:
                    bg.pop(0)()
                # output projection of this block runs inside the next
                # block's attention (bg queue); last block emits directly.
                if n == QB - 1:
                    for s in range(4 * n, 4 * n + 4):
                        emit_oproj(s)


_NC = None


def _get_program():
    global _NC
    if _NC is None:
        _NC = build_program()
    return _NC


def make_in_maps(query, key, value, Wq, bq, Wk, bk, Wv, bv, Wo):
    bf = ml_dtypes.bfloat16
    in_maps = []
    xqs = [np.ascontiguousarray(query[b].T).astype(bf) for b in range(B)]
    xks = [np.ascontiguousarray(key[b].T).astype(bf) for b in range(B)]
    xvs = [np.ascontiguousarray(value[b].T).astype(bf) for b in range(B)]
    for c in range(N_CORES):
        b, hg = c // 2, c % 2
        sl = slice(hg * DG, (hg + 1) * DG)
        in_maps.append({
            "xq": xqs[b], "xk": xks[b], "xv": xvs[b],
            "wq": np.ascontiguousarray(Wq[sl, :].T).astype(bf),
            "wk": np.ascontiguousarray(Wk[sl, :].T).astype(bf),
            "wv": np.ascontiguousarray(Wv[sl, :].T).astype(bf),
            "wo": np.ascontiguousarray(Wo[:, sl].T).astype(bf),
            "bq": np.asarray(bq[sl], np.float32).reshape(DG, 1),
            "bk": np.asarray(bk[sl], np.float32).reshape(DG, 1),
            "bv": np.asarray(bv[sl], np.float32).reshape(DG, 1),
        })
    return in_maps


def combine_outputs(results, bo):
    out = np.empty((B, S, D), np.float32)
    for b in range(B):
        out[b] = results[2 * b]["out"] + results[2 * b + 1]["out"]
        out[b] += np.asarray(bo, np.float32)[None, :]
    return out


def kernel(query, key, value, mask, Wq, bq, Wk, bk, Wv, bv, Wo, bo):
    # mask is the causal tril mask from the reference problem; causality is
    # implemented directly in the device kernel.
    nc = _get_program()
    in_maps = make_in_maps(
        np.asarray(query, np.float32), np.asarray(key, np.float32),
        np.asarray(value, np.float32), np.asarray(Wq, np.float32),
        np.asarray(bq, np.float32), np.asarray(Wk, np.float32),
        np.asarray(bk, np.float32), np.asarray(Wv, np.float32),
        np.asarray(bv, np.float32), np.asarray(Wo, np.float32),
    )
    res = run_bass_kernel_spmd(nc, in_maps, list(range(N_CORES)))
    return combine_outputs(res.results, np.asarray(bo, np.float32))

